# revision 23
# baseline (speedup 1.0000x reference)
"""CWTConvNet Trainium2 kernel.

The reference computes a 112-filter Morlet-wavelet SAME conv over length-2048
signals, then indexes the result with IMG_SELECT = linspace(0, 71, 224) cast
to int64 — i.e. only conv output positions 0..71 survive, each repeated 1-4
times. For those 72 positions only filter taps k in [209, 561) can touch
nonzero (non-pad) input, so the whole module reduces exactly to

    out72[f, s, l] = sum_{j=0}^{351} w2[f, j] * xe[s, j + l],   l in [0, 72)

with w2 = w_real[:, 0, 209:561] and xe = [71 zeros, x[s, 0:352], pad], then an
index-repeat expansion 72 -> 224 along the last axis.

Device kernel (per core, pure data parallel over 4 of 32 batches = 48
signals): the host supplies xe with groups of 12 signals interleaved
element-wise, so each im2col DMA descriptor carries 12 signals (1728B
contiguous runs — the im2col is descriptor-limited otherwise). Each group of
12 signals is an independent pipeline chain: 3 im2col DMAs (one per 128-tap
contraction chunk, all on the sync HWDGE ring so completions are FIFO),
2x3 accumulating matmuls into 2 PSUM banks, 2 plain PSUM->SBUF drains, and
one store on the scalar ring. The store keeps the (l, k)-interleaved PSUM
column order; the host undoes the interleave, applies the IMG_SELECT
repeat-gather, and unshards — all in one numpy pass.
"""

import numpy as np

import concourse.bacc as bacc
import concourse.bass as bass
import concourse.mybir as mybir
import concourse.tile as tile
from concourse.bass_utils import run_bass_kernel_spmd

# Problem constants (hardcoded; kernel.py must be self-contained).
B, C, L = 32, 12, 2048
F, K = 112, 561
NCORES = 8
BPC = B // NCORES          # batches per core
S = BPC * C                # signals per core (48)
NL = 72                    # conv output positions actually used
NI = 224                   # expanded output length
J = 352                    # taps that can touch non-pad input: k in [209, 561)
KOFF = 209                 # first needed tap
NCHUNK = 3                 # contraction chunks of 128 (352 -> 128,128,96)
XE_LEN = 456               # 71 zeros + 352 signal + tail zeros (>= 2*128+127+71+1)
XE_ZLEAD = 71

TI = 24                    # signals interleaved per im2col descriptor
NG = S // TI               # signal groups / pipeline chains per core (2)
NCOL_G = TI * NL           # matmul columns per group (1728)
NBANK = 4                  # PSUM banks per group (1728 fp32 cols)
NCOL_B = NCOL_G // NBANK   # columns per bank / matmul (432)
LPB = NL // NBANK          # l-positions per bank (18)

# Config: input dtype for the matmul operands. fp32 is exact; bf16 halves
# im2col DMA bytes and matmul passes at ~2e-3 relative error.
USE_BF16 = True

SEL = np.linspace(0, 71, NI, dtype=np.int64)

_CACHE = {}


def _build_nc():
    f32 = mybir.dt.float32
    dt_in = mybir.dt.bfloat16 if USE_BF16 else f32
    nc = bacc.Bacc("TRN2", target_bir_lowering=False, debug=False)

    # xg[g, t, k] = xe[12g + k, t]  (12-signal element interleave)
    xg_d = nc.declare_dram_parameter("xg", [NG, XE_LEN * TI], dt_in, isOutput=False)
    w_d = nc.declare_dram_parameter("w2t", [128, NCHUNK, F], dt_in, isOutput=False)
    # y[f, g, (l k)] keeps the interleaved PSUM column order; host undoes it.
    y_d = nc.declare_dram_parameter("y", [F, NG, NCOL_G], f32, isOutput=True)

    with tile.TileContext(nc) as tc:
        with (
            tc.tile_pool(name="sbuf", bufs=1) as pool,
            tc.tile_pool(name="psum", bufs=1, space="PSUM") as psum_pool,
        ):
            w_t = pool.tile([128, NCHUNK, F], dt_in, tag="w", name="w")
            nc.scalar.dma_start(out=w_t[:], in_=w_d.ap())

            psum_u = [
                psum_pool.tile([128, NCOL_B], f32, tag=f"ps{u}", name=f"ps{u}")
                for u in range(NG * NBANK)
            ]

            # im2col: rhs[p, (l k)] = xg[g, (128jc + p + l)*12 + k].
            # All on the sync ring: same-ring DMAs complete FIFO, so group 0's
            # chunks land first and its chain starts while later groups stream.
            rhs = {}
            for g in range(NG):
                for jc in range(NCHUNK):
                    r_t = pool.tile(
                        [128, NCOL_G], dt_in,
                        tag=f"rhs{g}_{jc}", name=f"rhs{g}_{jc}",
                    )
                    src = bass.AP(
                        tensor=xg_d,
                        offset=g * XE_LEN * TI + 128 * jc * TI,
                        ap=[[TI, 128], [1, NCOL_G]],
                    )
                    nc.sync.dma_start(out=r_t[:], in_=src)
                    rhs[(g, jc)] = r_t

            for g in range(NG):
                for jc in range(NCHUNK):
                    for b in range(NBANK):
                        nc.tensor.matmul(
                            psum_u[g * NBANK + b][:F, :],
                            w_t[:, jc, :],
                            rhs[(g, jc)][:, b * NCOL_B : (b + 1) * NCOL_B],
                            start=(jc == 0),
                            stop=(jc == NCHUNK - 1),
                        )
                # Plain contiguous drains (no de-interleave — host handles it)
                # on both PSUM-capable engines, then one store per bank pair.
                o72 = pool.tile([128, NCOL_G], f32, tag=f"o72_{g}", name=f"o72_{g}")
                for b in range(NBANK):
                    dst = o72[:F, b * NCOL_B : (b + 1) * NCOL_B]
                    if b % 2 == 0:
                        nc.scalar.copy(dst, psum_u[g * NBANK + b][:F, :])
                    else:
                        nc.vector.tensor_copy(out=dst, in_=psum_u[g * NBANK + b][:F, :])
                    if b % 2 == 1:
                        store_eng = nc.sync if g == 0 else nc.scalar
                        store_eng.dma_start(
                            out=y_d.ap()[:, g, (b - 1) * NCOL_B : (b + 1) * NCOL_B],
                            in_=o72[:F, (b - 1) * NCOL_B : (b + 1) * NCOL_B],
                        )

    nc.compile()
    return nc


def _get_nc():
    if "nc" not in _CACHE:
        _CACHE["nc"] = _build_nc()
    return _CACHE["nc"]


def _prepare_in_maps(x, w_real):
    if USE_BF16:
        import ml_dtypes

        np_in = np.dtype(ml_dtypes.bfloat16)
    else:
        np_in = np.dtype(np.float32)
    x = np.ascontiguousarray(np.asarray(x), dtype=np.float32)
    w_real = np.asarray(w_real, dtype=np.float32)

    w2t = np.zeros((NCHUNK * 128, F), np.float32)
    w2t[:J] = w_real[:, 0, KOFF:K].T
    w2t_dev = np.ascontiguousarray(
        w2t.reshape(NCHUNK, 128, F).transpose(1, 0, 2)
    ).astype(np_in)

    in_maps = []
    for m in range(NCORES):
        xe = np.zeros((S, XE_LEN), np.float32)
        xe[:, XE_ZLEAD : XE_ZLEAD + J] = x[m * BPC : (m + 1) * BPC].reshape(
            S, L
        )[:, :J]
        # interleave: xg[g, t, k] = xe[12g + k, t]
        xg = np.ascontiguousarray(
            xe.reshape(NG, TI, XE_LEN).transpose(0, 2, 1)
        ).reshape(NG, XE_LEN * TI)
        in_maps.append({"xg": xg.astype(np_in), "w2t": w2t_dev})
    return in_maps


def _assemble(results):
    # Device output: y[f, g, (l k)] with bank-major l split:
    # y[f, g, 432b + 12*lo + k] = out72[f, 12g + k, 36b + lo].
    ydev = np.stack([r["y"] for r in results])          # [8, F, NG, NCOL_G]
    yv = ydev.reshape(NCORES, F, NG, NBANK, LPB, TI)
    y72 = yv.transpose(0, 2, 5, 1, 3, 4)                # [8, NG, TI, F, NBANK, LPB]
    y72 = y72.reshape(NCORES, S, F, NL)                 # s = 12g + k, l = 36b + lo
    y = y72[..., SEL]                                   # [8, S, F, NI]
    return np.ascontiguousarray(y.reshape(B, C, F, NI))


def kernel(x, w_real):
    nc = _get_nc()
    in_maps = _prepare_in_maps(x, w_real)
    res = run_bass_kernel_spmd(nc, in_maps, list(range(NCORES)))
    return _assemble(res.results)


# revision 24
# speedup vs baseline: 1.0217x; 1.0217x over previous
"""CWTConvNet Trainium2 kernel.

The reference computes a 112-filter Morlet-wavelet SAME conv over length-2048
signals, then indexes the result with IMG_SELECT = linspace(0, 71, 224) cast
to int64 — i.e. only conv output positions 0..71 survive, each repeated 1-4
times. For those 72 positions only filter taps k in [209, 561) can touch
nonzero (non-pad) input, so the whole module reduces exactly to

    out72[f, s, l] = sum_{j=0}^{351} w2[f, j] * xe[s, j + l],   l in [0, 72)

with w2 = w_real[:, 0, 209:561] and xe = [71 zeros, x[s, 0:352], pad], then an
index-repeat expansion 72 -> 224 along the last axis.

Device kernel (per core, pure data parallel over 4 of 32 batches = 48
signals): the host supplies xe with groups of 12 signals interleaved
element-wise, so each im2col DMA descriptor carries 12 signals (1728B
contiguous runs — the im2col is descriptor-limited otherwise). Each group of
12 signals is an independent pipeline chain: 3 im2col DMAs (one per 128-tap
contraction chunk, all on the sync HWDGE ring so completions are FIFO),
2x3 accumulating matmuls into 2 PSUM banks, 2 plain PSUM->SBUF drains, and
one store on the scalar ring. The store keeps the (l, k)-interleaved PSUM
column order; the host undoes the interleave, applies the IMG_SELECT
repeat-gather, and unshards — all in one numpy pass.
"""

import numpy as np

import concourse.bacc as bacc
import concourse.bass as bass
import concourse.mybir as mybir
import concourse.tile as tile
from concourse.bass_utils import run_bass_kernel_spmd

# Problem constants (hardcoded; kernel.py must be self-contained).
B, C, L = 32, 12, 2048
F, K = 112, 561
NCORES = 8
BPC = B // NCORES          # batches per core
S = BPC * C                # signals per core (48)
NL = 72                    # conv output positions actually used
NI = 224                   # expanded output length
J = 352                    # taps that can touch non-pad input: k in [209, 561)
KOFF = 209                 # first needed tap
NCHUNK = 3                 # contraction chunks of 128 (352 -> 128,128,96)
XE_LEN = 456               # 71 zeros + 352 signal + tail zeros (>= 2*128+127+71+1)
XE_ZLEAD = 71

TI = 24                    # signals interleaved per im2col descriptor
NG = S // TI               # signal groups / pipeline chains per core (2)
NCOL_G = TI * NL           # matmul columns per group (1728)
NBANK = 4                  # PSUM banks per group (1728 fp32 cols)
NCOL_B = NCOL_G // NBANK   # columns per bank / matmul (432)
LPB = NL // NBANK          # l-positions per bank (18)

# Config: input dtype for the matmul operands. fp32 is exact; bf16 halves
# im2col DMA bytes and matmul passes at ~2e-3 relative error.
USE_BF16 = True

SEL = np.linspace(0, 71, NI, dtype=np.int64)

_CACHE = {}


def _build_nc():
    f32 = mybir.dt.float32
    dt_in = mybir.dt.bfloat16 if USE_BF16 else f32
    nc = bacc.Bacc("TRN2", target_bir_lowering=False, debug=False)

    # xg[g, t, k] = xe[12g + k, t]  (12-signal element interleave)
    xg_d = nc.declare_dram_parameter("xg", [NG, XE_LEN * TI], dt_in, isOutput=False)
    w_d = nc.declare_dram_parameter("w2t", [128, NCHUNK, F], dt_in, isOutput=False)
    # y[f, g, (l k)] keeps the interleaved PSUM column order; host undoes it.
    y_d = nc.declare_dram_parameter("y", [F, NG, NCOL_G], f32, isOutput=True)

    with tile.TileContext(nc) as tc:
        with (
            tc.tile_pool(name="sbuf", bufs=1) as pool,
            tc.tile_pool(name="psum", bufs=1, space="PSUM") as psum_pool,
        ):
            w_t = pool.tile([128, NCHUNK, F], dt_in, tag="w", name="w")
            nc.scalar.dma_start(out=w_t[:], in_=w_d.ap())

            psum_u = [
                psum_pool.tile([128, NCOL_B], f32, tag=f"ps{u}", name=f"ps{u}")
                for u in range(NG * NBANK)
            ]

            # im2col: rhs[p, (l k)] = xg[g, (128jc + p + l)*12 + k].
            # All on the sync ring: same-ring DMAs complete FIFO, so group 0's
            # chunks land first and its chain starts while later groups stream.
            rhs = {}
            for g in range(NG):
                for jc in range(NCHUNK):
                    r_t = pool.tile(
                        [128, NCOL_G], dt_in,
                        tag=f"rhs{g}_{jc}", name=f"rhs{g}_{jc}",
                    )
                    src = bass.AP(
                        tensor=xg_d,
                        offset=g * XE_LEN * TI + 128 * jc * TI,
                        ap=[[TI, 128], [1, NCOL_G]],
                    )
                    nc.sync.dma_start(out=r_t[:], in_=src)
                    rhs[(g, jc)] = r_t

            for g in range(NG):
                for jc in range(NCHUNK):
                    for b in range(NBANK):
                        nc.tensor.matmul(
                            psum_u[g * NBANK + b][:F, :],
                            w_t[:, jc, :],
                            rhs[(g, jc)][:, b * NCOL_B : (b + 1) * NCOL_B],
                            start=(jc == 0),
                            stop=(jc == NCHUNK - 1),
                        )
                # Plain contiguous drains (no de-interleave — host handles it)
                # on both PSUM-capable engines, then one store per PSUM bank.
                o72 = pool.tile([128, NCOL_G], f32, tag=f"o72_{g}", name=f"o72_{g}")
                for b in range(NBANK):
                    dst = o72[:F, b * NCOL_B : (b + 1) * NCOL_B]
                    if (g + b) % 2 == 0:
                        nc.scalar.copy(dst, psum_u[g * NBANK + b][:F, :])
                    else:
                        nc.vector.tensor_copy(out=dst, in_=psum_u[g * NBANK + b][:F, :])
                    nc.sync.dma_start(
                        out=y_d.ap()[:, g, b * NCOL_B : (b + 1) * NCOL_B],
                        in_=dst,
                    )

    nc.compile()
    return nc


def _get_nc():
    if "nc" not in _CACHE:
        _CACHE["nc"] = _build_nc()
    return _CACHE["nc"]


def _prepare_in_maps(x, w_real):
    if USE_BF16:
        import ml_dtypes

        np_in = np.dtype(ml_dtypes.bfloat16)
    else:
        np_in = np.dtype(np.float32)
    x = np.ascontiguousarray(np.asarray(x), dtype=np.float32)
    w_real = np.asarray(w_real, dtype=np.float32)

    w2t = np.zeros((NCHUNK * 128, F), np.float32)
    w2t[:J] = w_real[:, 0, KOFF:K].T
    w2t_dev = np.ascontiguousarray(
        w2t.reshape(NCHUNK, 128, F).transpose(1, 0, 2)
    ).astype(np_in)

    in_maps = []
    for m in range(NCORES):
        xe = np.zeros((S, XE_LEN), np.float32)
        xe[:, XE_ZLEAD : XE_ZLEAD + J] = x[m * BPC : (m + 1) * BPC].reshape(
            S, L
        )[:, :J]
        # interleave: xg[g, t, k] = xe[12g + k, t]
        xg = np.ascontiguousarray(
            xe.reshape(NG, TI, XE_LEN).transpose(0, 2, 1)
        ).reshape(NG, XE_LEN * TI)
        in_maps.append({"xg": xg.astype(np_in), "w2t": w2t_dev})
    return in_maps


def _assemble(results):
    # Device output: y[f, g, (l k)] with bank-major l split:
    # y[f, g, 432b + 12*lo + k] = out72[f, 12g + k, 36b + lo].
    ydev = np.stack([r["y"] for r in results])          # [8, F, NG, NCOL_G]
    yv = ydev.reshape(NCORES, F, NG, NBANK, LPB, TI)
    y72 = yv.transpose(0, 2, 5, 1, 3, 4)                # [8, NG, TI, F, NBANK, LPB]
    y72 = y72.reshape(NCORES, S, F, NL)                 # s = 12g + k, l = 36b + lo
    y = y72[..., SEL]                                   # [8, S, F, NI]
    return np.ascontiguousarray(y.reshape(B, C, F, NI))


def kernel(x, w_real):
    nc = _get_nc()
    in_maps = _prepare_in_maps(x, w_real)
    res = run_bass_kernel_spmd(nc, in_maps, list(range(NCORES)))
    return _assemble(res.results)
